# revision 51
# baseline (speedup 1.0000x reference)
"""Multi-head self-attention on 8 Trainium2 NeuronCores.

Strategy (batch x head-group sharding):
  - 2 batches x 4 head-groups -> each core owns batch b = core//4 and
    heads 4g..4g+3 (g = core%4): a 256-column slice of Wq/Wk/Wv and the
    matching 256-row slice of Wo, applied to one batch's tokens.
  - All matmul operands are bf16 (cast on the host): full PE rate and
    half the SBUF/DMA traffic of fp32.  (fp8 was numerically tested and
    rejected: e4m3 scores -> 3.1e-2 rel err, over the 2e-2 gate.
    Schraudolph-exp offload was tried and removed: the DVE serialization
    it causes costs more than the ACT time it saves.)
  - Q/K/V are projected in [e, s] layout (weights stationary); V is
    then flipped to [s, e] via the DMA xbar transpose engine, then
    strided DVE copies into the ones-augmented AV layout [V_h | 1].
  - Scores are computed transposed, ST[k, q] = K^T Q, two heads
    row-packed into the PE array (64-wide contraction per head).
  - exp on ACT (table exp, bf16 out); the softmax denominator rides the
    AV matmul via the ones column.  Per-head normalization happens on
    the transposed attention matrix right before the output projection;
    partial outputs are written bf16 and the 4 per-batch partials are
    summed on host (the Wo row-parallel all-reduce) with bo added there.

Schedule (engines execute in emission order, so placement == schedule):
  Inputs stream over three queues (sync HWDGE, scalar HWDGE, gpsimd
  SWDGE) in first-use order; weights are sl-split so only the sl0
  halves sit on the critical path.  The lead-in projects just K/Q for
  the first 512 tokens (1.5MB critical prefix) and block 0 starts ~15us
  in.  Most blocks are ACT-paced (16 exps ~ 18.6us), so projection and
  output-projection work is spread in ~0.43us quarter/half items across
  block slots to keep the PE stream matched to the exp stream (psS is
  only double-buffered - any burstiness stalls one engine or the
  other).  AV matmuls trail the exp stream by `lag` k-tiles.
  Consecutive blocks are software-pipelined: block N+1's kt0 scores+exp
  are emitted before block N's tail-AV drain.
PSUM: scores 2x[128,1024] (4 banks) + AV accumulators 2x[65,512]
  (2 banks) + proj/outproj 2x[128,512] (2 banks) = 8.
"""
import sys

sys.path.insert(0, "/opt/trn_rl_repo")

import numpy as np
import ml_dtypes

import concourse.bacc as bacc
import concourse.tile as tile
from concourse import mybir
from concourse.bass_utils import run_bass_kernel_spmd

AF = mybir.ActivationFunctionType
F32 = mybir.dt.float32
BF = mybir.dt.bfloat16
BF_NP = ml_dtypes.bfloat16

N_CORES = 8
D = 1024          # model dim
S = 2048          # tokens per core (one batch)
E = 256           # per-core projection width (4 heads x 64)
HD = 64           # head dim
P = 128           # partitions
QC = 512          # q-chunk
SC = 1024         # projection s-chunk
DC = D // P       # 8
N_KT = S // P     # 16
N_QC = S // QC    # 4
EW = HD + 1       # per-head V width with ones column


def build_attention_core(with_qkv_bias=False):
    scale = 1.0 / np.sqrt(np.float32(HD))

    nc = bacc.Bacc("TRN2", target_bir_lowering=False)
    xT = nc.dram_tensor("xT", [P, DC, S], BF, kind="ExternalInput")
    wq = nc.dram_tensor("wq", [P, 2, DC, P], BF, kind="ExternalInput")
    wk = nc.dram_tensor("wk", [P, 2, DC, P], BF, kind="ExternalInput")
    wv = nc.dram_tensor("wv", [P, 2, DC, P], BF, kind="ExternalInput")
    wo = nc.dram_tensor("wo", [P, 2, D], BF, kind="ExternalInput")
    bq = nc.dram_tensor("bq", [P, 2], F32, kind="ExternalInput")
    bk = nc.dram_tensor("bk", [P, 2], F32, kind="ExternalInput")
    bv = nc.dram_tensor("bv", [P, 2], F32, kind="ExternalInput")
    out = nc.dram_tensor("out", [S, D], BF, kind="ExternalOutput")

    with tile.TileContext(nc) as tc:
        with (
            tc.tile_pool(name="persist", bufs=1) as persist,
            tc.tile_pool(name="attp", bufs=6) as attp,
            tc.tile_pool(name="upool", bufs=8) as upool,
            tc.tile_pool(name="vtrp", bufs=2) as vtrp,
            tc.tile_pool(name="pafp", bufs=2) as pafp,
            tc.tile_pool(name="small", bufs=2) as small,
            tc.tile_pool(name="outp", bufs=2) as outp,
            tc.tile_pool(name="psS", bufs=2, space="PSUM") as psS,
            tc.tile_pool(name="psP", bufs=2, space="PSUM") as psP,
            tc.tile_pool(name="psQ", bufs=2, space="PSUM") as psQ,
        ):
            w_sb = {}
            for nm in ("k", "v", "q"):
                w_sb[nm] = persist.tile([P, 2, DC, P], BF, tag=f"w_{nm}",
                                        name=f"w_{nm}")
            wo_sb = persist.tile([P, 2, D], BF)
            x_sb = persist.tile([P, DC, S], BF)

            # ---- input DMA: three queues, first-use order ----------------
            # Queue discipline: ACT carries only input issues then the exp
            # stream; gpsimd only early input issues (its queue must stay
            # clear for tail broadcasts); sync carries inputs, then vflip
            # transposes, then output stores.  Critical prefix: wk_sl0,
            # wq_sl0, x[:, :, 0:512] (1.5MB).
            # scalar/ACT queue: ONLY the three critical weight halves —
            # every queued DMA issue on this engine delays the first exp.
            nc.scalar.dma_start(w_sb["k"][:, 0], wk[:, 0])
            nc.scalar.dma_start(w_sb["q"][:, 0], wq[:, 0])
            nc.scalar.dma_start(w_sb["v"][:, 0], wv[:, 0])
            nc.sync.dma_start(x_sb[:, 0:4, 0:512], xT[:, 0:4, 0:512])
            nc.sync.dma_start(x_sb[:, 0:4, 512:SC], xT[:, 0:4, 512:SC])
            nc.sync.dma_start(x_sb[:, 0:4, SC:2 * SC], xT[:, 0:4, SC:2 * SC])
            nc.sync.dma_start(w_sb["k"][:, 1], wk[:, 1])
            nc.sync.dma_start(w_sb["q"][:, 1], wq[:, 1])
            nc.sync.dma_start(w_sb["v"][:, 1], wv[:, 1])
            nc.sync.dma_start(wo_sb[:], wo[:])
            nc.gpsimd.dma_start(x_sb[:, 4:8, 0:512], xT[:, 4:8, 0:512])
            nc.gpsimd.dma_start(x_sb[:, 4:8, 512:SC], xT[:, 4:8, 512:SC])
            nc.gpsimd.dma_start(x_sb[:, 4:8, SC:2 * SC],
                                xT[:, 4:8, SC:2 * SC])

            bias_t = {}
            if with_qkv_bias:
                for nm, t in (("q", bq), ("k", bk), ("v", bv)):
                    bt = persist.tile([P, 2], F32, tag=f"b_{nm}")
                    nc.gpsimd.dma_start(bt[:], t[:])
                    bias_t[nm] = bt

            # ---- persistent activations ----------------------------------
            KT = persist.tile([P, 2, S], BF, tag="KT")   # [e, slice, s]
            QT = persist.tile([P, 2, S], BF, tag="QT")
            VT = persist.tile([P, 2, S], BF, tag="VT")
            # AV stationary: per k-chunk [V_h0|1|V_h1|1|V_h2|1|V_h3|1]
            V_sb = persist.tile([P, N_KT, 4 * EW], BF, tag="V")
            V_r = V_sb[:].rearrange("p c (h u) -> p c h u", u=EW)
            for h in range(4):
                nc.gpsimd.memset(V_r[:, :, h, HD], 1.0)

            # ---- projection emitters (o-range items) ---------------------
            dsts = {"k": KT, "q": QT, "v": VT}
            ctx = {}

            def proj_piece(nm, sl, s0, olo, ohi, slen=SC):
                """Matmuls for o in [olo, ohi); alloc at olo==0, PSUM->SBUF
                copy at ohi==DC.  PSUM is per-512 tiles from psQ."""
                key = (nm, sl, s0)
                if olo == 0:
                    ctx[key] = [psQ.tile([P, 512], F32, tag="Q",
                                         name=f"ps_{nm}{hh}")
                                for hh in range(slen // 512)]
                pss = ctx[key]
                for o in range(olo, ohi):
                    for hh in range(slen // 512):
                        nc.tensor.matmul(
                            pss[hh][:],
                            w_sb[nm][:, sl, o, :],
                            x_sb[:, o, s0 + hh * 512:s0 + (hh + 1) * 512],
                            start=(o == 0), stop=(o == DC - 1),
                        )
                if ohi == DC:
                    del ctx[key]
                    for hh in range(slen // 512):
                        dst = dsts[nm][:, sl,
                                       s0 + hh * 512:s0 + (hh + 1) * 512]
                        if with_qkv_bias:
                            nc.vector.tensor_tensor(
                                dst, pss[hh][:],
                                bias_t[nm][:, sl:sl + 1]
                                .to_broadcast((P, 512)),
                                mybir.AluOpType.add)
                        else:
                            nc.vector.tensor_copy(dst, pss[hh][:])

            def PJ(nm, sl, s0, part, slen=SC):
                """Quarter item: o-range [2*part, 2*part+2)."""
                return lambda: proj_piece(nm, sl, s0, 2 * part,
                                          2 * part + 2, slen)

            def H1(nm, sl, s0, slen=SC):
                return lambda: proj_piece(nm, sl, s0, 0, 4, slen)

            def H2(nm, sl, s0, slen=SC):
                return lambda: proj_piece(nm, sl, s0, 4, 8, slen)

            def emit_vflip(h, sc):
                """Transpose head h's V tokens [sc*SC,(sc+1)*SC) into V_sb."""
                sl, h2 = divmod(h, 2)
                vtr = vtrp.tile([P, SC // P, HD], BF, tag="vtr")
                nc.sync.dma_start_transpose(
                    vtr[:],
                    VT[h2 * HD:(h2 + 1) * HD, sl, sc * SC:(sc + 1) * SC])
                c0 = sc * (SC // P)
                nc.vector.tensor_copy(
                    V_r[:, c0:c0 + SC // P, h, 0:HD], vtr[:])

            F = lambda h, sc: (lambda: emit_vflip(h, sc))

            # ---- attention -----------------------------------------------
            def emit_scores_exp(p, qc, kt):
                """Scores + exp for one k-tile; returns a pend entry."""
                q0 = qc * QC
                k0 = kt * P
                st = psS.tile([P, 2 * QC], F32, tag="S", name="st")
                nc.tensor.matmul(
                    st[:, 0:QC],
                    KT[0:HD, p, k0:k0 + P], QT[0:HD, p, q0:q0 + QC],
                    tile_position=(0, 0), start=True, stop=True)
                nc.tensor.matmul(
                    st[:, QC:2 * QC],
                    KT[HD:P, p, k0:k0 + P], QT[HD:P, p, q0:q0 + QC],
                    tile_position=(64, 0), start=True, stop=True)
                ut = upool.tile([P, 2 * QC], BF, tag="U")
                nc.scalar.activation(ut[:], st[:], AF.Exp,
                                     scale=float(scale))
                return (kt, ut)

            def emit_block(p, qc, sched, lag, catchup,
                           head=(), next_head_fn=None):
                """Scores+exp+AV for head pair p, q-chunk qc.

                sched: {kt: [callables]} -- projection/outproj work emitted
                into that kt slot ('post' runs after the AV drain).  AV
                trails exp by `lag` k-tiles (catching up from kt=catchup).
                head: pend entries pre-emitted by the previous block.
                next_head_fn: emits the next block's kt0 just before this
                block's tail-AV drain (software pipelining).
                """
                pa = [psP.tile([EW, QC], F32, tag="P", name=f"pa{h}")
                      for h in range(2)]

                def emit_av(kt, ut):
                    for h in range(2):
                        nc.tensor.matmul(
                            pa[h][:],
                            V_sb[:, kt, (2 * p + h) * EW:(2 * p + h + 1) * EW],
                            ut[:, h * QC:(h + 1) * QC],
                            start=(kt == 0), stop=(kt == N_KT - 1))

                pend = list(head)
                for kt in range(N_KT):
                    if kt >= len(head):
                        pend.append(emit_scores_exp(p, qc, kt))
                    hi = kt - lag
                    if catchup is not None:
                        hi += max(0, kt - catchup)
                    while pend and pend[0][0] <= hi:
                        emit_av(*pend.pop(0))
                    for fn in sched.get(kt, ()):
                        fn()
                nh = next_head_fn() if next_head_fn else ()
                for item in pend:
                    emit_av(*item)
                for fn in sched.get('post', ()):
                    fn()
                return pa, nh

            def emit_tail(p, qc, pa, final=False):
                """Normalize pair p's attention -> attnT (bf16, persists).

                Non-final: pa is copied to SBUF immediately (releasing the
                PSUM accumulators for the next block's AVs — the long
                normalize chain would otherwise stall them and reset the PE
                p-state) and the normalize runs off the critical path.
                """
                rinv1 = small.tile([1, 2 * QC], F32, tag="rinv1")
                if final:
                    src = pa
                    s_of = lambda h: slice(0, QC)
                    rsb = small.tile([1, 2 * QC], F32, tag="rsb")
                    for h in range(2):
                        nc.vector.tensor_copy(
                            rsb[0:1, h * QC:(h + 1) * QC], pa[h][HD:EW, :])
                    nc.vector.reciprocal_approx_fast(rinv1[:], rsb[:])
                else:
                    paf = pafp.tile([EW, 2 * QC], F32, tag="paf")
                    for h in range(2):
                        nc.vector.tensor_copy(
                            paf[:, h * QC:(h + 1) * QC], pa[h][:])
                    src = [paf, paf]
                    s_of = lambda h: slice(h * QC, (h + 1) * QC)
                    rsb = small.tile([1, 2 * QC], F32, tag="rsb")
                    nc.vector.tensor_copy(rsb[0:1, :], paf[HD:EW, :])
                    nc.vector.reciprocal_approx_fast(rinv1[:], rsb[:])
                rb = small.tile([HD, 2 * QC], F32, tag="rb")
                nc.gpsimd.partition_broadcast(rb[:], rinv1[0:1, :])
                attnT = attp.tile([P, QC], BF, tag=f"attnT_{p}_{qc}")
                for h in range(2):
                    nc.vector.tensor_tensor(
                        attnT[h * HD:(h + 1) * HD, :],
                        src[h][0:HD, s_of(h)], rb[:, h * QC:(h + 1) * QC],
                        mybir.AluOpType.mult)
                return attnT

            attnT = {}
            _fin_q = [0]

            def out_dma(dst, src, late=False):
                # Mid-run stores ride the otherwise-idle sync queue; the
                # final stores (exp stream over) alternate sync/scalar.
                if late:
                    eng = nc.sync if _fin_q[0] % 2 == 0 else nc.scalar
                    _fin_q[0] += 1
                else:
                    eng = nc.sync
                eng.dma_start(dst, src)

            def oph(qc, ss, oc, late=False):
                """Half of one outproj unit: one 512-wide oc column."""
                key = ('osb', qc, ss)
                if oc == 0:
                    ctx[key] = outp.tile([P, D], BF, tag="osb", name="osb")
                po = psQ.tile([P, 512], F32, tag="Q", name="po")
                aT = (attnT[(0, qc)], attnT[(1, qc)])
                for p in range(2):
                    nc.tensor.matmul(
                        po[:], aT[p][:, ss * P:(ss + 1) * P],
                        wo_sb[:, p, oc * 512:(oc + 1) * 512],
                        start=(p == 0), stop=(p == 1))
                osb = ctx[key]
                nc.vector.tensor_copy(osb[:, oc * 512:(oc + 1) * 512], po[:])
                if oc == 1:
                    del ctx[key]
                    q0 = qc * QC
                    out_dma(out[q0 + ss * P:q0 + (ss + 1) * P, :], osb[:],
                            late=late)

            def OP(qc, ss, oc, late=False):
                return lambda: oph(qc, ss, oc, late)

            def emit_outproj_final_A(qc):
                # Pair-0 partial accumulation for row chunks 0-2 — fills the
                # PE while the final normalize chain runs on DVE/GPSIMD.
                # (ss0/ss1 in the freed score buffers, ss2 in psQ; ss3 would
                # deadlock on pool reuse and runs fully in phase B.)
                aT0 = attnT[(0, qc)]
                pos = {}
                for ss in range(2):
                    po = psS.tile([P, SC], F32, tag="S", name="po_f")
                    for oc in range(D // 512):
                        nc.tensor.matmul(
                            po[:, oc * 512:(oc + 1) * 512],
                            aT0[:, ss * P:(ss + 1) * P],
                            wo_sb[:, 0, oc * 512:(oc + 1) * 512],
                            start=True, stop=False)
                    pos[ss] = [po, po]
                pos[2] = []
                for oc in range(D // 512):
                    po = psQ.tile([P, 512], F32, tag="Q", name="po_f2")
                    nc.tensor.matmul(
                        po[:], aT0[:, 2 * P:3 * P],
                        wo_sb[:, 0, oc * 512:(oc + 1) * 512],
                        start=True, stop=False)
                    pos[2].append(po)
                return pos

            def emit_outproj_final_B(qc, pos):
                q0 = qc * QC
                aT1 = attnT[(1, qc)]

                def fin(ss, po_of):
                    osb = outp.tile([P, D], BF, tag="osb", name="osb_f")
                    for oc in range(D // 512):
                        po, of = po_of[oc]
                        nc.tensor.matmul(
                            po[:, of:of + 512],
                            aT1[:, ss * P:(ss + 1) * P],
                            wo_sb[:, 1, oc * 512:(oc + 1) * 512],
                            start=False, stop=True)
                        nc.vector.tensor_copy(
                            osb[:, oc * 512:(oc + 1) * 512],
                            po[:, of:of + 512])
                    out_dma(out[q0 + ss * P:q0 + (ss + 1) * P, :], osb[:],
                            late=True)

                for ss in range(2):
                    fin(ss, [(pos[ss][oc], oc * 512) for oc in range(2)])
                fin(2, [(pos[2][oc], 0) for oc in range(2)])
                # ss3: full unit; psS buffer reuse is safe now (ss0's copies
                # are already emitted above).
                po = psS.tile([P, SC], F32, tag="S", name="po_f3")
                aT0 = attnT[(0, qc)]
                for oc in range(D // 512):
                    nc.tensor.matmul(
                        po[:, oc * 512:(oc + 1) * 512],
                        aT0[:, 3 * P:4 * P],
                        wo_sb[:, 0, oc * 512:(oc + 1) * 512],
                        start=True, stop=False)
                fin(3, [(po, oc * 512) for oc in range(2)])

            # ---- schedule ------------------------------------------------
            # Lead-in: only the first 512 tokens of K and Q — the minimum
            # needed for B0's kt0 scores.
            proj_piece("k", 0, 0, 0, DC, 512)
            proj_piece("q", 0, 0, 0, DC, 512)

            # blocks: (p, qc, sched, lag, catchup).  Work is spread in
            # ~0.43us quarter/half items to match the ACT exp pace; the
            # thin blocks (B1/B2/B6/B7) stay ACT-paced at ~18.6us.
            blocks = [
                # B0: rest of K00/V00, K01/V01 as x(sc1) lands, Q00b late.
                # lag 6: AVs start right after the sc0 flips are emitted.
                (0, 0, {0: [H1("k", 0, 512, 512)], 1: [H2("k", 0, 512, 512)],
                        2: [H1("v", 0, 0)], 3: [H2("v", 0, 0)],
                        4: [F(0, 0)], 5: [F(1, 0)],
                        6: [H1("k", 0, SC)], 7: [H2("k", 0, SC)],
                        8: [H1("v", 0, SC)], 9: [H2("v", 0, SC)],
                        10: [F(0, 1)], 11: [F(1, 1)],
                        12: [H1("q", 0, 512, 512)],
                        13: [H2("q", 0, 512, 512)]},
                 6, 9),
                # B1: Q01 (for B2/B3) in quarters.
                (0, 1, {0: [PJ("q", 0, SC, 0)], 1: [PJ("q", 0, SC, 1)],
                        2: [PJ("q", 0, SC, 2)], 3: [PJ("q", 0, SC, 3)]},
                 5, 8),
                # B2: K10 + V10 (+flips, early so no AV ever waits on them).
                (0, 2, {0: [PJ("k", 1, 0, 0)], 1: [PJ("k", 1, 0, 1)],
                        2: [PJ("k", 1, 0, 2)], 3: [PJ("k", 1, 0, 3)],
                        4: [PJ("v", 1, 0, 0)], 5: [PJ("v", 1, 0, 1)],
                        6: [PJ("v", 1, 0, 2)], 7: [PJ("v", 1, 0, 3)],
                        8: [F(2, 0)], 9: [F(3, 0)]},
                 5, 8),
                # B3: Q10 + V11 (+flips).
                (0, 3, {0: [PJ("q", 1, 0, 0)], 1: [PJ("q", 1, 0, 1)],
                        2: [PJ("q", 1, 0, 2)], 3: [PJ("q", 1, 0, 3)],
                        4: [PJ("v", 1, SC, 0)], 5: [PJ("v", 1, SC, 1)],
                        6: [PJ("v", 1, SC, 2)], 7: [PJ("v", 1, SC, 3)],
                        8: [F(2, 1)], 9: [F(3, 1)]},
                 5, 8),
                # B4: K11 only (its V/flips were produced in B2/B3).
                (1, 0, {0: [PJ("k", 1, SC, 0)], 1: [PJ("k", 1, SC, 1)],
                        2: [PJ("k", 1, SC, 2)], 3: [PJ("k", 1, SC, 3)]},
                 5, 8),
                # B5: Q11 + outproj(qc0) halves.
                (1, 1, {0: [PJ("q", 1, SC, 0)], 1: [PJ("q", 1, SC, 1)],
                        2: [PJ("q", 1, SC, 2)], 3: [PJ("q", 1, SC, 3)],
                        5: [OP(0, 0, 0)], 6: [OP(0, 0, 1)],
                        7: [OP(0, 1, 0)], 8: [OP(0, 1, 1)],
                        9: [OP(0, 2, 0)], 10: [OP(0, 2, 1)],
                        11: [OP(0, 3, 0)], 12: [OP(0, 3, 1)]},
                 5, 8),
                # B6: outproj(qc1) halves (first one waits B5's attnT).
                (1, 2, {4: [OP(1, 0, 0)], 5: [OP(1, 0, 1)],
                        7: [OP(1, 1, 0)], 8: [OP(1, 1, 1)],
                        10: [OP(1, 2, 0)], 11: [OP(1, 2, 1)],
                        13: [OP(1, 3, 0)], 14: [OP(1, 3, 1)]},
                 5, 8),
                # B7: outproj(qc2) halves; last unit after the AV drain to
                # cover the final normalize.
                (1, 3, {4: [OP(2, 0, 0)], 5: [OP(2, 0, 1)],
                        7: [OP(2, 1, 0)], 8: [OP(2, 1, 1)],
                        10: [OP(2, 2, 0)], 11: [OP(2, 2, 1)],
                        'post': [OP(2, 3, 0, True), OP(2, 3, 1, True)]},
                 5, 8),
            ]

            head = ()
            for i, (p, qc, sched, lag, cu) in enumerate(blocks):
                if i + 1 < len(blocks):
                    np_, nqc = blocks[i + 1][0], blocks[i + 1][1]

                    def next_head_fn(np_=np_, nqc=nqc):
                        return [emit_scores_exp(np_, nqc, 0),
                                emit_scores_exp(np_, nqc, 1)]
                else:
                    next_head_fn = None
                pa, head = emit_block(p, qc, sched, lag, cu,
                                      head=head, next_head_fn=next_head_fn)
                if i == len(blocks) - 1:
                    # Final: the pair-0 half of qc3's outproj runs on the PE
                    # while the last normalize chain runs on DVE/GPSIMD.
                    po_fin = emit_outproj_final_A(3)
                    attnT[(p, qc)] = emit_tail(p, qc, pa, final=True)
                    emit_outproj_final_B(3, po_fin)
                else:
                    attnT[(p, qc)] = emit_tail(p, qc, pa)

    nc.compile()
    return nc


_NC_CACHE = {}


def _get_nc(with_qkv_bias):
    key = with_qkv_bias
    if key not in _NC_CACHE:
        _NC_CACHE[key] = build_attention_core(with_qkv_bias)
    return _NC_CACHE[key]


def _pack_pdm(a):
    """[D, M] -> [128, D//128, M] partition-major, bf16."""
    Dd, M = a.shape
    return np.ascontiguousarray(
        a.reshape(Dd // P, P, M).transpose(1, 0, 2).astype(BF_NP))


def _pack_w(a):
    """[D, E] -> [128, 2, D//128, 128]: w[p, sl, o, j] = W[o*128+p, sl*128+j]."""
    Dd, Ee = a.shape
    return np.ascontiguousarray(
        a.reshape(Dd // P, P, Ee // P, P).transpose(1, 2, 0, 3).astype(BF_NP))


def run_attention(x, Wq, bq, Wk, bk, Wv, bv, Wo, bo, trace=False):
    B, S_, D_ = x.shape
    assert (B, S_, D_) == (2, S, D)
    with_qkv_bias = bool(np.any(bq) or np.any(bk) or np.any(bv))
    nc = _get_nc(with_qkv_bias)
    in_maps = []
    for c in range(N_CORES):
        b, g = divmod(c, N_CORES // 2)
        sl = slice(g * E, (g + 1) * E)
        xTb = np.ascontiguousarray(x[b].T)  # [D, S]
        in_maps.append({
            "xT": _pack_pdm(xTb),
            "wq": _pack_w(Wq[:, sl]),
            "wk": _pack_w(Wk[:, sl]),
            "wv": _pack_w(Wv[:, sl]),
            "wo": np.ascontiguousarray(
                Wo[sl, :].reshape(2, P, D).transpose(1, 0, 2)
                .astype(BF_NP)),
            "bq": np.ascontiguousarray(
                bq[sl].reshape(2, P).T.astype(np.float32)),
            "bk": np.ascontiguousarray(
                bk[sl].reshape(2, P).T.astype(np.float32)),
            "bv": np.ascontiguousarray(
                bv[sl].reshape(2, P).T.astype(np.float32)),
        })
    res = run_bass_kernel_spmd(nc, in_maps, core_ids=list(range(N_CORES)),
                               trace=trace)
    outs = []
    for b in range(2):
        acc = np.zeros((S, D), dtype=np.float32)
        for g in range(N_CORES // 2):
            acc += np.asarray(res.results[b * 4 + g]["out"]).astype(np.float32)
        outs.append(acc + np.asarray(bo, dtype=np.float32)[None, :])
    return np.stack(outs).reshape(B, S, D), res


def kernel(x, Wq, bq, Wk, bk, Wv, bv, Wo, bo):
    out, _ = run_attention(np.asarray(x), np.asarray(Wq), np.asarray(bq),
                           np.asarray(Wk), np.asarray(bk), np.asarray(Wv),
                           np.asarray(bv), np.asarray(Wo), np.asarray(bo))
    return out


# revision 54
# speedup vs baseline: 1.0007x; 1.0007x over previous
"""Multi-head self-attention on 8 Trainium2 NeuronCores.

Strategy (batch x head-group sharding):
  - 2 batches x 4 head-groups -> each core owns batch b = core//4 and
    heads 4g..4g+3 (g = core%4): a 256-column slice of Wq/Wk/Wv and the
    matching 256-row slice of Wo, applied to one batch's tokens.
  - All matmul operands are bf16 (cast on the host): full PE rate and
    half the SBUF/DMA traffic of fp32.  (fp8 was numerically tested and
    rejected: e4m3 scores -> 3.1e-2 rel err, over the 2e-2 gate.
    Schraudolph-exp offload was tried and removed: the DVE serialization
    it causes costs more than the ACT time it saves.)
  - Q/K/V are projected in [e, s] layout (weights stationary); V is
    then flipped to [s, e] via the DMA xbar transpose engine, then
    strided DVE copies into the ones-augmented AV layout [V_h | 1].
  - Scores are computed transposed, ST[k, q] = K^T Q, two heads
    row-packed into the PE array (64-wide contraction per head).
  - exp on ACT (table exp, bf16 out); the softmax denominator rides the
    AV matmul via the ones column.  Per-head normalization happens on
    the transposed attention matrix right before the output projection;
    partial outputs are written bf16 and the 4 per-batch partials are
    summed on host (the Wo row-parallel all-reduce) with bo added there.

Schedule (engines execute in emission order, so placement == schedule):
  Inputs stream over three queues (sync HWDGE, scalar HWDGE, gpsimd
  SWDGE) in first-use order; weights are sl-split so only the sl0
  halves sit on the critical path.  The lead-in projects just K/Q for
  the first 512 tokens (1.5MB critical prefix) and block 0 starts ~15us
  in.  Most blocks are ACT-paced (16 exps ~ 18.6us), so projection and
  output-projection work is spread in ~0.43us quarter/half items across
  block slots to keep the PE stream matched to the exp stream (psS is
  only double-buffered - any burstiness stalls one engine or the
  other).  AV matmuls trail the exp stream by `lag` k-tiles.
  Consecutive blocks are software-pipelined: block N+1's kt0 scores+exp
  are emitted before block N's tail-AV drain.
PSUM: scores 2x[128,1024] (4 banks) + AV accumulators 2x[65,512]
  (2 banks) + proj/outproj 2x[128,512] (2 banks) = 8.
"""
import sys

sys.path.insert(0, "/opt/trn_rl_repo")

import numpy as np
import ml_dtypes

import concourse.bacc as bacc
import concourse.tile as tile
from concourse import mybir
from concourse.bass_utils import run_bass_kernel_spmd

AF = mybir.ActivationFunctionType
F32 = mybir.dt.float32
BF = mybir.dt.bfloat16
BF_NP = ml_dtypes.bfloat16

N_CORES = 8
D = 1024          # model dim
S = 2048          # tokens per core (one batch)
E = 256           # per-core projection width (4 heads x 64)
HD = 64           # head dim
P = 128           # partitions
QC = 512          # q-chunk
SC = 1024         # projection s-chunk
DC = D // P       # 8
N_KT = S // P     # 16
N_QC = S // QC    # 4
EW = HD + 1       # per-head V width with ones column


def build_attention_core(with_qkv_bias=False):
    scale = 1.0 / np.sqrt(np.float32(HD))

    nc = bacc.Bacc("TRN2", target_bir_lowering=False)
    xT = nc.dram_tensor("xT", [P, DC, S], BF, kind="ExternalInput")
    wq = nc.dram_tensor("wq", [P, 2, DC, P], BF, kind="ExternalInput")
    wk = nc.dram_tensor("wk", [P, 2, DC, P], BF, kind="ExternalInput")
    wv = nc.dram_tensor("wv", [P, 2, DC, P], BF, kind="ExternalInput")
    wo = nc.dram_tensor("wo", [P, 2, D], BF, kind="ExternalInput")
    bq = nc.dram_tensor("bq", [P, 2], F32, kind="ExternalInput")
    bk = nc.dram_tensor("bk", [P, 2], F32, kind="ExternalInput")
    bv = nc.dram_tensor("bv", [P, 2], F32, kind="ExternalInput")
    out = nc.dram_tensor("out", [S, D], BF, kind="ExternalOutput")

    with tile.TileContext(nc) as tc:
        with (
            tc.tile_pool(name="persist", bufs=1) as persist,
            tc.tile_pool(name="attp", bufs=6) as attp,
            tc.tile_pool(name="upool", bufs=8) as upool,
            tc.tile_pool(name="vtrp", bufs=2) as vtrp,
            tc.tile_pool(name="pafp", bufs=2) as pafp,
            tc.tile_pool(name="small", bufs=2) as small,
            tc.tile_pool(name="outp", bufs=2) as outp,
            tc.tile_pool(name="psS", bufs=2, space="PSUM") as psS,
            tc.tile_pool(name="psP", bufs=2, space="PSUM") as psP,
            tc.tile_pool(name="psQ", bufs=2, space="PSUM") as psQ,
        ):
            w_sb = {}
            for nm in ("k", "v", "q"):
                w_sb[nm] = persist.tile([P, 2, DC, P], BF, tag=f"w_{nm}",
                                        name=f"w_{nm}")
            wo_sb = persist.tile([P, 2, D], BF)
            x_sb = persist.tile([P, DC, S], BF)

            # ---- input DMA: three queues, first-use order ----------------
            # Queue discipline: ACT carries only input issues then the exp
            # stream; gpsimd only early input issues (its queue must stay
            # clear for tail broadcasts); sync carries inputs, then vflip
            # transposes, then output stores.  Critical prefix: wk_sl0,
            # wq_sl0, x[:, :, 0:512] (1.5MB).
            # scalar/ACT queue: ONLY the three critical weight halves —
            # every queued DMA issue on this engine delays the first exp.
            nc.scalar.dma_start(w_sb["k"][:, 0], wk[:, 0])
            nc.scalar.dma_start(w_sb["q"][:, 0], wq[:, 0])
            nc.scalar.dma_start(w_sb["v"][:, 0], wv[:, 0])
            nc.sync.dma_start(x_sb[:, 0:4, 0:512], xT[:, 0:4, 0:512])
            nc.sync.dma_start(x_sb[:, 0:4, 512:SC], xT[:, 0:4, 512:SC])
            nc.sync.dma_start(x_sb[:, 0:4, SC:2 * SC], xT[:, 0:4, SC:2 * SC])
            nc.sync.dma_start(w_sb["k"][:, 1], wk[:, 1])
            nc.sync.dma_start(w_sb["q"][:, 1], wq[:, 1])
            nc.sync.dma_start(w_sb["v"][:, 1], wv[:, 1])
            nc.sync.dma_start(wo_sb[:], wo[:])
            nc.gpsimd.dma_start(x_sb[:, 4:8, 0:512], xT[:, 4:8, 0:512])
            nc.gpsimd.dma_start(x_sb[:, 4:8, 512:SC], xT[:, 4:8, 512:SC])
            nc.gpsimd.dma_start(x_sb[:, 4:8, SC:2 * SC],
                                xT[:, 4:8, SC:2 * SC])

            bias_t = {}
            if with_qkv_bias:
                for nm, t in (("q", bq), ("k", bk), ("v", bv)):
                    bt = persist.tile([P, 2], F32, tag=f"b_{nm}")
                    nc.gpsimd.dma_start(bt[:], t[:])
                    bias_t[nm] = bt

            # ---- persistent activations ----------------------------------
            KT = persist.tile([P, 2, S], BF, tag="KT")   # [e, slice, s]
            QT = persist.tile([P, 2, S], BF, tag="QT")
            VT = persist.tile([P, 2, S], BF, tag="VT")
            # AV stationary: per k-chunk [V_h0|1|V_h1|1|V_h2|1|V_h3|1]
            V_sb = persist.tile([P, N_KT, 4 * EW], BF, tag="V")
            V_r = V_sb[:].rearrange("p c (h u) -> p c h u", u=EW)
            for h in range(4):
                nc.gpsimd.memset(V_r[:, :, h, HD], 1.0)

            # ---- projection emitters (o-range items) ---------------------
            dsts = {"k": KT, "q": QT, "v": VT}
            ctx = {}

            def proj_piece(nm, sl, s0, olo, ohi, slen=SC):
                """Matmuls for o in [olo, ohi); alloc at olo==0, PSUM->SBUF
                copy at ohi==DC.  PSUM is per-512 tiles from psQ."""
                key = (nm, sl, s0)
                if olo == 0:
                    ctx[key] = [psQ.tile([P, 512], F32, tag="Q",
                                         name=f"ps_{nm}{hh}")
                                for hh in range(slen // 512)]
                pss = ctx[key]
                for o in range(olo, ohi):
                    for hh in range(slen // 512):
                        nc.tensor.matmul(
                            pss[hh][:],
                            w_sb[nm][:, sl, o, :],
                            x_sb[:, o, s0 + hh * 512:s0 + (hh + 1) * 512],
                            start=(o == 0), stop=(o == DC - 1),
                        )
                if ohi == DC:
                    del ctx[key]
                    for hh in range(slen // 512):
                        dst = dsts[nm][:, sl,
                                       s0 + hh * 512:s0 + (hh + 1) * 512]
                        if with_qkv_bias:
                            nc.vector.tensor_tensor(
                                dst, pss[hh][:],
                                bias_t[nm][:, sl:sl + 1]
                                .to_broadcast((P, 512)),
                                mybir.AluOpType.add)
                        else:
                            nc.vector.tensor_copy(dst, pss[hh][:])

            def PJ(nm, sl, s0, part, slen=SC):
                """Quarter item: o-range [2*part, 2*part+2)."""
                return lambda: proj_piece(nm, sl, s0, 2 * part,
                                          2 * part + 2, slen)

            def H1(nm, sl, s0, slen=SC):
                return lambda: proj_piece(nm, sl, s0, 0, 4, slen)

            def H2(nm, sl, s0, slen=SC):
                return lambda: proj_piece(nm, sl, s0, 4, 8, slen)

            def emit_vflip(h, sc):
                """Transpose head h's V tokens [sc*SC,(sc+1)*SC) into V_sb."""
                sl, h2 = divmod(h, 2)
                vtr = vtrp.tile([P, SC // P, HD], BF, tag="vtr")
                nc.sync.dma_start_transpose(
                    vtr[:],
                    VT[h2 * HD:(h2 + 1) * HD, sl, sc * SC:(sc + 1) * SC])
                c0 = sc * (SC // P)
                nc.vector.tensor_copy(
                    V_r[:, c0:c0 + SC // P, h, 0:HD], vtr[:])

            F = lambda h, sc: (lambda: emit_vflip(h, sc))

            # ---- attention -----------------------------------------------
            def emit_scores_exp(p, qc, kt):
                """Scores + exp for one k-tile; returns a pend entry."""
                q0 = qc * QC
                k0 = kt * P
                st = psS.tile([P, 2 * QC], F32, tag="S", name="st")
                nc.tensor.matmul(
                    st[:, 0:QC],
                    KT[0:HD, p, k0:k0 + P], QT[0:HD, p, q0:q0 + QC],
                    tile_position=(0, 0), start=True, stop=True)
                nc.tensor.matmul(
                    st[:, QC:2 * QC],
                    KT[HD:P, p, k0:k0 + P], QT[HD:P, p, q0:q0 + QC],
                    tile_position=(64, 0), start=True, stop=True)
                ut = upool.tile([P, 2 * QC], BF, tag="U")
                nc.scalar.activation(ut[:], st[:], AF.Exp,
                                     scale=float(scale))
                return (kt, ut)

            def emit_block(p, qc, sched, lag, catchup,
                           head=(), next_head_fn=None):
                """Scores+exp+AV for head pair p, q-chunk qc.

                sched: {kt: [callables]} -- projection/outproj work emitted
                into that kt slot ('post' runs after the AV drain).  AV
                trails exp by `lag` k-tiles (catching up from kt=catchup).
                head: pend entries pre-emitted by the previous block.
                next_head_fn: emits the next block's kt0 just before this
                block's tail-AV drain (software pipelining).
                """
                pa = [psP.tile([EW, QC], F32, tag="P", name=f"pa{h}")
                      for h in range(2)]

                def emit_av(kt, ut):
                    for h in range(2):
                        nc.tensor.matmul(
                            pa[h][:],
                            V_sb[:, kt, (2 * p + h) * EW:(2 * p + h + 1) * EW],
                            ut[:, h * QC:(h + 1) * QC],
                            start=(kt == 0), stop=(kt == N_KT - 1))

                pend = list(head)
                for kt in range(N_KT):
                    if kt >= len(head):
                        pend.append(emit_scores_exp(p, qc, kt))
                    hi = kt - lag
                    if catchup is not None:
                        hi += max(0, kt - catchup)
                    while pend and pend[0][0] <= hi:
                        emit_av(*pend.pop(0))
                    for fn in sched.get(kt, ()):
                        fn()
                nh = next_head_fn() if next_head_fn else ()
                for item in pend:
                    emit_av(*item)
                for fn in sched.get('post', ()):
                    fn()
                return pa, nh

            def emit_tail(p, qc, pa, final=False):
                """Normalize pair p's attention -> attnT (bf16, persists).

                Non-final: pa is copied to SBUF immediately (releasing the
                PSUM accumulators for the next block's AVs — the long
                normalize chain would otherwise stall them and reset the PE
                p-state) and the normalize runs off the critical path.
                """
                rinv1 = small.tile([1, 2 * QC], F32, tag="rinv1")
                if final:
                    src = pa
                    s_of = lambda h: slice(0, QC)
                    rsb = small.tile([1, 2 * QC], F32, tag="rsb")
                    for h in range(2):
                        nc.vector.tensor_copy(
                            rsb[0:1, h * QC:(h + 1) * QC], pa[h][HD:EW, :])
                    nc.vector.reciprocal_approx_fast(rinv1[:], rsb[:])
                else:
                    paf = pafp.tile([EW, 2 * QC], F32, tag="paf")
                    for h in range(2):
                        nc.vector.tensor_copy(
                            paf[:, h * QC:(h + 1) * QC], pa[h][:])
                    src = [paf, paf]
                    s_of = lambda h: slice(h * QC, (h + 1) * QC)
                    rsb = small.tile([1, 2 * QC], F32, tag="rsb")
                    nc.vector.tensor_copy(rsb[0:1, :], paf[HD:EW, :])
                    nc.vector.reciprocal_approx_fast(rinv1[:], rsb[:])
                rb = small.tile([HD, 2 * QC], F32, tag="rb")
                nc.gpsimd.partition_broadcast(rb[:], rinv1[0:1, :])
                attnT = attp.tile([P, QC], BF, tag=f"attnT_{p}_{qc}")
                for h in range(2):
                    nc.vector.tensor_tensor(
                        attnT[h * HD:(h + 1) * HD, :],
                        src[h][0:HD, s_of(h)], rb[:, h * QC:(h + 1) * QC],
                        mybir.AluOpType.mult)
                return attnT

            attnT = {}
            _fin_q = [0]

            def out_dma(dst, src, late=False):
                # Mid-run stores ride the otherwise-idle sync queue; the
                # final stores (exp stream over) alternate sync/scalar.
                if late:
                    eng = nc.sync if _fin_q[0] % 2 == 0 else nc.scalar
                    _fin_q[0] += 1
                else:
                    eng = nc.sync
                eng.dma_start(dst, src)

            def oph(qc, ss, oc, late=False):
                """Half of one outproj unit: one 512-wide oc column."""
                key = ('osb', qc, ss)
                if oc == 0:
                    ctx[key] = outp.tile([P, D], BF, tag="osb", name="osb")
                po = psQ.tile([P, 512], F32, tag="Q", name="po")
                aT = (attnT[(0, qc)], attnT[(1, qc)])
                for p in range(2):
                    nc.tensor.matmul(
                        po[:], aT[p][:, ss * P:(ss + 1) * P],
                        wo_sb[:, p, oc * 512:(oc + 1) * 512],
                        start=(p == 0), stop=(p == 1))
                osb = ctx[key]
                nc.vector.tensor_copy(osb[:, oc * 512:(oc + 1) * 512], po[:])
                if oc == 1:
                    del ctx[key]
                    q0 = qc * QC
                    out_dma(out[q0 + ss * P:q0 + (ss + 1) * P, :], osb[:],
                            late=late)

            def OP(qc, ss, oc, late=False):
                return lambda: oph(qc, ss, oc, late)

            def emit_outproj_final_A(qc):
                # Pair-0 partial accumulation for row chunks 0-2 — fills the
                # PE while the final normalize chain runs on DVE/GPSIMD.
                # (ss0/ss1 in the freed score buffers, ss2 in psQ; ss3 would
                # deadlock on pool reuse and runs fully in phase B.)
                aT0 = attnT[(0, qc)]
                pos = {}
                for ss in range(2):
                    po = psS.tile([P, SC], F32, tag="S", name="po_f")
                    for oc in range(D // 512):
                        nc.tensor.matmul(
                            po[:, oc * 512:(oc + 1) * 512],
                            aT0[:, ss * P:(ss + 1) * P],
                            wo_sb[:, 0, oc * 512:(oc + 1) * 512],
                            start=True, stop=False)
                    pos[ss] = [po, po]
                return pos

            def emit_outproj_final_B(qc, pos):
                q0 = qc * QC
                aT1 = attnT[(1, qc)]

                def fin(ss, po_of):
                    osb = outp.tile([P, D], BF, tag="osb", name="osb_f")
                    for oc in range(D // 512):
                        po, of = po_of[oc]
                        nc.tensor.matmul(
                            po[:, of:of + 512],
                            aT1[:, ss * P:(ss + 1) * P],
                            wo_sb[:, 1, oc * 512:(oc + 1) * 512],
                            start=False, stop=True)
                        nc.vector.tensor_copy(
                            osb[:, oc * 512:(oc + 1) * 512],
                            po[:, of:of + 512])
                    out_dma(out[q0 + ss * P:q0 + (ss + 1) * P, :], osb[:],
                            late=True)

                for ss in range(2):
                    fin(ss, [(pos[ss][oc], oc * 512) for oc in range(2)])
                # ss2/ss3: full units pipelined through psP (free once the
                # final normalize has consumed the pa accumulators).
                aT0 = attnT[(0, qc)]
                for ss in (2, 3):
                    osb = outp.tile([P, D], BF, tag="osb", name="osb_f")
                    for oc in range(D // 512):
                        po = psP.tile([P, 512], F32, tag="P", name="po2")
                        for p, aT in ((0, aT0), (1, aT1)):
                            nc.tensor.matmul(
                                po[:], aT[:, ss * P:(ss + 1) * P],
                                wo_sb[:, p, oc * 512:(oc + 1) * 512],
                                start=(p == 0), stop=(p == 1))
                        nc.vector.tensor_copy(
                            osb[:, oc * 512:(oc + 1) * 512], po[:])
                    out_dma(out[q0 + ss * P:q0 + (ss + 1) * P, :], osb[:],
                            late=True)

            # ---- schedule ------------------------------------------------
            # Lead-in: only the first 512 tokens of K and Q — the minimum
            # needed for B0's kt0 scores.
            proj_piece("k", 0, 0, 0, DC, 512)
            proj_piece("q", 0, 0, 0, DC, 512)

            # blocks: (p, qc, sched, lag, catchup).  Work is spread in
            # ~0.43us quarter/half items to match the ACT exp pace; the
            # thin blocks (B1/B2/B6/B7) stay ACT-paced at ~18.6us.
            blocks = [
                # B0: rest of K00/V00, K01/V01 as x(sc1) lands, Q00b late.
                # lag 6: AVs start right after the sc0 flips are emitted.
                (0, 0, {0: [H1("k", 0, 512, 512)], 1: [H2("k", 0, 512, 512)],
                        2: [H1("v", 0, 0)], 3: [H2("v", 0, 0)],
                        4: [H1("k", 0, SC)], 5: [H2("k", 0, SC)],
                        6: [F(0, 0)], 7: [F(1, 0)],
                        8: [H1("v", 0, SC)], 9: [H2("v", 0, SC)],
                        10: [F(0, 1)], 11: [F(1, 1)],
                        12: [H1("q", 0, 512, 512)],
                        13: [H2("q", 0, 512, 512)]},
                 8, None),
                # B1: Q01 (for B2/B3) in quarters.
                (0, 1, {0: [PJ("q", 0, SC, 0)], 1: [PJ("q", 0, SC, 1)],
                        2: [PJ("q", 0, SC, 2)], 3: [PJ("q", 0, SC, 3)]},
                 5, 8),
                # B2: K10 + V10 (+flips, early so no AV ever waits on them).
                (0, 2, {0: [PJ("k", 1, 0, 0)], 1: [PJ("k", 1, 0, 1)],
                        2: [PJ("k", 1, 0, 2)], 3: [PJ("k", 1, 0, 3)],
                        4: [PJ("v", 1, 0, 0)], 5: [PJ("v", 1, 0, 1)],
                        6: [PJ("v", 1, 0, 2)], 7: [PJ("v", 1, 0, 3)],
                        8: [F(2, 0)], 9: [F(3, 0)]},
                 5, 8),
                # B3: Q10 + V11 (+flips).
                (0, 3, {0: [PJ("q", 1, 0, 0)], 1: [PJ("q", 1, 0, 1)],
                        2: [PJ("q", 1, 0, 2)], 3: [PJ("q", 1, 0, 3)],
                        4: [PJ("v", 1, SC, 0)], 5: [PJ("v", 1, SC, 1)],
                        6: [PJ("v", 1, SC, 2)], 7: [PJ("v", 1, SC, 3)],
                        8: [F(2, 1)], 9: [F(3, 1)]},
                 5, 8),
                # B4: K11 only (its V/flips were produced in B2/B3).
                (1, 0, {0: [PJ("k", 1, SC, 0)], 1: [PJ("k", 1, SC, 1)],
                        2: [PJ("k", 1, SC, 2)], 3: [PJ("k", 1, SC, 3)]},
                 5, 8),
                # B5: Q11 + outproj(qc0) halves.
                (1, 1, {0: [PJ("q", 1, SC, 0)], 1: [PJ("q", 1, SC, 1)],
                        2: [PJ("q", 1, SC, 2)], 3: [PJ("q", 1, SC, 3)],
                        5: [OP(0, 0, 0)], 6: [OP(0, 0, 1)],
                        7: [OP(0, 1, 0)], 8: [OP(0, 1, 1)],
                        9: [OP(0, 2, 0)], 10: [OP(0, 2, 1)],
                        11: [OP(0, 3, 0)], 12: [OP(0, 3, 1)]},
                 5, 8),
                # B6: outproj(qc1) halves (first one waits B5's attnT).
                (1, 2, {4: [OP(1, 0, 0)], 5: [OP(1, 0, 1)],
                        7: [OP(1, 1, 0)], 8: [OP(1, 1, 1)],
                        10: [OP(1, 2, 0)], 11: [OP(1, 2, 1)],
                        13: [OP(1, 3, 0)], 14: [OP(1, 3, 1)]},
                 5, 8),
                # B7: outproj(qc2) halves; last unit after the AV drain to
                # cover the final normalize.
                (1, 3, {4: [OP(2, 0, 0)], 5: [OP(2, 0, 1)],
                        7: [OP(2, 1, 0)], 8: [OP(2, 1, 1)],
                        10: [OP(2, 2, 0)], 11: [OP(2, 2, 1)],
                        'post': [OP(2, 3, 0, True), OP(2, 3, 1, True)]},
                 5, 8),
            ]

            head = ()
            for i, (p, qc, sched, lag, cu) in enumerate(blocks):
                if i + 1 < len(blocks):
                    np_, nqc = blocks[i + 1][0], blocks[i + 1][1]

                    def next_head_fn(np_=np_, nqc=nqc):
                        return [emit_scores_exp(np_, nqc, 0),
                                emit_scores_exp(np_, nqc, 1)]
                else:
                    next_head_fn = None
                pa, head = emit_block(p, qc, sched, lag, cu,
                                      head=head, next_head_fn=next_head_fn)
                if i == len(blocks) - 1:
                    # Final: the pair-0 half of qc3's outproj runs on the PE
                    # while the last normalize chain runs on DVE/GPSIMD.
                    po_fin = emit_outproj_final_A(3)
                    attnT[(p, qc)] = emit_tail(p, qc, pa, final=True)
                    emit_outproj_final_B(3, po_fin)
                else:
                    attnT[(p, qc)] = emit_tail(p, qc, pa)

    nc.compile()
    return nc


_NC_CACHE = {}


def _get_nc(with_qkv_bias):
    key = with_qkv_bias
    if key not in _NC_CACHE:
        _NC_CACHE[key] = build_attention_core(with_qkv_bias)
    return _NC_CACHE[key]


def _pack_pdm(a):
    """[D, M] -> [128, D//128, M] partition-major, bf16."""
    Dd, M = a.shape
    return np.ascontiguousarray(
        a.reshape(Dd // P, P, M).transpose(1, 0, 2).astype(BF_NP))


def _pack_w(a):
    """[D, E] -> [128, 2, D//128, 128]: w[p, sl, o, j] = W[o*128+p, sl*128+j]."""
    Dd, Ee = a.shape
    return np.ascontiguousarray(
        a.reshape(Dd // P, P, Ee // P, P).transpose(1, 2, 0, 3).astype(BF_NP))


def run_attention(x, Wq, bq, Wk, bk, Wv, bv, Wo, bo, trace=False):
    B, S_, D_ = x.shape
    assert (B, S_, D_) == (2, S, D)
    with_qkv_bias = bool(np.any(bq) or np.any(bk) or np.any(bv))
    nc = _get_nc(with_qkv_bias)
    in_maps = []
    for c in range(N_CORES):
        b, g = divmod(c, N_CORES // 2)
        sl = slice(g * E, (g + 1) * E)
        xTb = np.ascontiguousarray(x[b].T)  # [D, S]
        in_maps.append({
            "xT": _pack_pdm(xTb),
            "wq": _pack_w(Wq[:, sl]),
            "wk": _pack_w(Wk[:, sl]),
            "wv": _pack_w(Wv[:, sl]),
            "wo": np.ascontiguousarray(
                Wo[sl, :].reshape(2, P, D).transpose(1, 0, 2)
                .astype(BF_NP)),
            "bq": np.ascontiguousarray(
                bq[sl].reshape(2, P).T.astype(np.float32)),
            "bk": np.ascontiguousarray(
                bk[sl].reshape(2, P).T.astype(np.float32)),
            "bv": np.ascontiguousarray(
                bv[sl].reshape(2, P).T.astype(np.float32)),
        })
    res = run_bass_kernel_spmd(nc, in_maps, core_ids=list(range(N_CORES)),
                               trace=trace)
    outs = []
    for b in range(2):
        acc = np.zeros((S, D), dtype=np.float32)
        for g in range(N_CORES // 2):
            acc += np.asarray(res.results[b * 4 + g]["out"]).astype(np.float32)
        outs.append(acc + np.asarray(bo, dtype=np.float32)[None, :])
    return np.stack(outs).reshape(B, S, D), res


def kernel(x, Wq, bq, Wk, bk, Wv, bv, Wo, bo):
    out, _ = run_attention(np.asarray(x), np.asarray(Wq), np.asarray(bq),
                           np.asarray(Wk), np.asarray(bk), np.asarray(Wv),
                           np.asarray(bv), np.asarray(Wo), np.asarray(bo))
    return out
